# revision 1
# baseline (speedup 1.0000x reference)
"""Bahdanau attention scorer on 8 NeuronCores (Trainium2, Bass/Tile).

scores[t,b,s] = sum_a v_a[a] * tanh( (W_s @ enc[s,b])[a] + (W_t @ dec[t,b])[a] + b_t[a] )

Sharding: data-parallel over batch (32 -> 4 per core); W_s/W_t/b_t/v_a replicated.

Per-core dataflow (BC=4 batch elems):
  - DMA enc/dec slices in [s,h]-major layout (2KB contiguous rows).
  - PE-transpose 128x128 blocks to get h on partitions; project with
    pre-transposed weights: E = W_s @ enc_b^T -> [a=128, s=256] (likewise D, +b_t).
  - Main loop over (b, t-group): DVE/GpSimd tensor_scalar adds E + D[:,t]
    (per-partition scalar) into a [128, TG*256] staging tile; single ACT tanh
    over the tile; per (t, s-block) a PE matmul with lhsT = tanh chunk [a,128],
    rhs = v_a [a,1] writes a dense [s,1] PSUM column, accumulating a
    [s=128, t=256] score tile per s-block.
  - PE-transpose score tiles to [t, s] and DMA out as [128, 256] blocks
    (1KB contiguous per partition row).
"""

import os

import numpy as np

SRC, TRG, BATCH, HID, ATT = 256, 256, 32, 512, 128
N_CORES = 8
BC = BATCH // N_CORES  # batch elems per core
TG = int(os.environ.get("K_TG", "16"))  # t's per staging group
SB_BUFS = int(os.environ.get("K_SB_BUFS", "3"))

_NC_CACHE = {}


def build_nc(trg=TRG, tanh_dt=None, reps=1):
    import concourse.tile as tile
    from concourse import bacc, mybir

    f32 = mybir.dt.float32
    if tanh_dt is None:
        # fp16 matvec operands: fp32 PE matmuls are multi-pass (~445ns/op vs
        # 58ns for 16-bit, HW-measured); tanh in [-1,1] keeps fp16 error ~6e-4
        tanh_dt = {
            "f32": mybir.dt.float32,
            "fp16": mybir.dt.float16,
            "bf16": mybir.dt.bfloat16,
        }[os.environ.get("K_TANH_DT", "fp16")]
    tg = min(TG, trg)
    assert trg % tg == 0

    nc = bacc.Bacc(
        "TRN2", target_bir_lowering=False, debug=False, num_devices=N_CORES
    )
    dec_in = nc.dram_tensor("dec_out", [trg, BC, HID], f32, kind="ExternalInput")
    enc_in = nc.dram_tensor("enc_outs", [SRC, BC, HID], f32, kind="ExternalInput")
    ws_in = nc.dram_tensor("W_s", [ATT, HID], f32, kind="ExternalInput")
    wt_in = nc.dram_tensor("W_t", [ATT, HID], f32, kind="ExternalInput")
    bt_in = nc.dram_tensor("b_t", [ATT, 1], f32, kind="ExternalInput")
    va_in = nc.dram_tensor("v_a", [ATT, 1], f32, kind="ExternalInput")
    id_in = nc.dram_tensor("ident128", [128, 128], f32, kind="ExternalInput")
    nonce_in = None
    if reps > 1:
        # shape-distinct dummy input so no compile cache can confuse
        # reps-variants of this program
        nonce_in = nc.dram_tensor("nonce", [reps, 16], f32, kind="ExternalInput")
    out = nc.dram_tensor("scores", [trg, BC, SRC], f32, kind="ExternalOutput")

    NHB = HID // 128  # h blocks
    NSB = SRC // 128  # s blocks
    TANH = mybir.ActivationFunctionType.Tanh

    with tile.TileContext(nc) as tc:
        with (
            tc.tile_pool(name="consts", bufs=1) as consts,
            tc.tile_pool(name="wraw", bufs=1) as wraw,
            tc.tile_pool(name="raw", bufs=4) as raw,
            tc.tile_pool(name="enct", bufs=2) as enct,
            tc.tile_pool(name="ed", bufs=2 if reps > 1 else 1) as ed,
            tc.tile_pool(name="sums", bufs=SB_BUFS) as sums_pool,
            tc.tile_pool(name="tanh", bufs=SB_BUFS) as tanh_pool,
            tc.tile_pool(name="scout", bufs=3) as scout_pool,
            tc.tile_pool(name="otile", bufs=3) as otile_pool,
            tc.tile_pool(name="tp_ps", bufs=3, space="PSUM") as tp_ps,
            tc.tile_pool(name="proj_ps", bufs=2, space="PSUM") as proj_ps,
            tc.tile_pool(name="sc_ps", bufs=3, space="PSUM") as sc_ps,
        ):
            # identity comes in as an input: cheaper + off the gpsimd
            # critical path vs make_identity
            ident = consts.tile([128, 128], f32)
            nc.sync.dma_start(out=ident[:], in_=id_in[:])
            # warm the ACT tanh table-load off the critical path
            warm = consts.tile([1, 2], f32)
            nc.vector.memset(warm[:], 0.0)
            nc.scalar.activation(warm[:], warm[:], TANH)
            v_sb = consts.tile([128, 1], tanh_dt)
            if tanh_dt == f32:
                nc.sync.dma_start(out=v_sb[:], in_=va_in[:])
            else:
                v_f32 = consts.tile([128, 1], f32)
                nc.sync.dma_start(out=v_f32[:], in_=va_in[:])
                nc.vector.tensor_copy(v_sb[:], v_f32[:])
            bt_sb = consts.tile([128, 1], f32)
            nc.sync.dma_start(out=bt_sb[:], in_=bt_in[:])
            if nonce_in is not None:
                nonce_sb = consts.tile([reps, 16], f32)
                nc.sync.dma_start(out=nonce_sb[:], in_=nonce_in[:])

            # --- transpose weights: W [a, h] -> WT blocks [h128, a128] ---
            wT = {}
            for name, w_in in (("s", ws_in), ("t", wt_in)):
                w_sb = wraw.tile([128, HID], f32, tag="wsb", name=f"w{name}raw")
                nc.sync.dma_start(out=w_sb[:], in_=w_in[:])
                wT[name] = consts.tile(
                    [128, NHB, 128], f32, tag=f"w{name}T", name=f"w{name}T"
                )
                for hb in range(NHB):
                    ps = tp_ps.tile([128, 128], f32, tag="tp", name=f"tpw{name}{hb}")
                    nc.tensor.transpose(
                        ps[:], w_sb[:, hb * 128 : (hb + 1) * 128], ident[:]
                    )
                    nc.vector.tensor_copy(wT[name][:, hb, :], ps[:])

            probe0 = os.environ.get("K_PROBE", "")
            for rep in range(reps):
                # --- per-b: load enc/dec, transpose, project to E/D [a, s|t] ---
                if probe0 == "nosetup" and rep > 0:
                    pass  # reuse rep-0's E_sb/D_sb
                else:
                    E_sb = ed.tile([128, BC, SRC], f32, tag="E", name=f"E_{rep}")
                    D_sb = ed.tile([128, BC, trg], f32, tag="D", name=f"D_{rep}")
                setup_bs = (
                    [] if (probe0 == "nosetup" and rep > 0) else list(range(BC))
                )
                for b in setup_bs:
                    for name, src_dram, n_free, ed_dst in (
                        ("enc", enc_in, SRC, E_sb),
                        ("dec", dec_in, trg, D_sb),
                    ):
                        nfb = n_free // 128
                        xT = enct.tile(
                            [128, NHB, n_free], f32, tag="xT", name=f"xT{rep}_{b}{name}"
                        )
                        for fb in range(nfb):
                            x_raw = raw.tile(
                                [128, HID], f32, tag="raw", name=f"raw{rep}_{b}{name}{fb}"
                            )
                            nc.sync.dma_start(
                                out=x_raw[:],
                                in_=src_dram[fb * 128 : (fb + 1) * 128, b, :],
                            )
                            for hb in range(NHB):
                                ps = tp_ps.tile(
                                    [128, 128], f32, tag="tp", name=f"tp{rep}_{b}{name}{fb}{hb}"
                                )
                                nc.tensor.transpose(
                                    ps[:], x_raw[:, hb * 128 : (hb + 1) * 128], ident[:]
                                )
                                nc.vector.tensor_copy(
                                    xT[:, hb, fb * 128 : (fb + 1) * 128], ps[:]
                                )
                        w_key = "s" if name == "enc" else "t"
                        pps = proj_ps.tile(
                            [128, n_free], f32, tag="proj", name=f"proj{rep}_{b}{name}"
                        )
                        for hb in range(NHB):
                            nc.tensor.matmul(
                                pps[:],
                                wT[w_key][:, hb, :],
                                xT[:, hb, :],
                                start=(hb == 0),
                                stop=(hb == NHB - 1),
                            )
                        if name == "enc":
                            nc.vector.tensor_copy(ed_dst[:, b, :], pps[:])
                        else:
                            # D = W_t @ dec^T + b_t (bias folded into PSUM->SBUF copy)
                            nc.vector.tensor_scalar_add(
                                ed_dst[:, b, :], pps[:], bt_sb[:, 0:1]
                            )

                # --- main loop ---
                probe = os.environ.get("K_PROBE", "")
                if probe == "onlysetup":
                    # touch E/D so release checks pass; skip main loop
                    sink = scout_pool.tile([128, 8], f32, tag="scsb", name=f"sink{rep}")
                    nc.vector.tensor_copy(sink[:, 0:4], E_sb[:, 0, 0:4])
                    nc.vector.tensor_copy(sink[:, 4:8], D_sb[:, 0, 0:4])
                    ot = otile_pool.tile([128, SRC], f32, tag="ot", name=f"oto{rep}")
                    nc.vector.memset(ot[:], 0.0)
                    nc.sync.dma_start(out=out[0:128, 0, :], in_=ot[:])
                    continue
                stg_shared = None
                if probe == "noadds":
                    stg_shared = sums_pool.tile(
                        [128, tg * SRC], f32, tag="sums", name=f"stgsh{rep}"
                    )
                    nc.vector.memset(stg_shared[:], 0.1)
                for b in range(BC):
                    sc_psum = [
                        sc_ps.tile([128, trg], f32, tag="sc", name=f"sc{rep}_{b}_{sb}")
                        for sb in range(NSB)
                    ]
                    if probe == "nomm":
                        for sb in range(NSB):
                            nc.vector.memset(sc_psum[sb][:], 0.0)
                    for g in range(trg // tg):
                        # last FUSED of every tg t's go through ACT directly
                        # (bias+tanh in one op); the rest are DVE adds + one
                        # batched ACT tanh — balances DVE and ACT.
                        FUSED = 0 if probe else 4
                        nb = tg - FUSED
                        if probe == "noadds":
                            stg = stg_shared
                        else:
                            stg = sums_pool.tile(
                                [128, nb * SRC], f32, tag="sums", name=f"stg{rep}_{b}_{g}"
                            )
                            for j in range(nb):
                                t = g * tg + j
                                nc.vector.tensor_scalar_add(
                                    stg[:, j * SRC : (j + 1) * SRC],
                                    E_sb[:, b, :],
                                    D_sb[:, b, t : t + 1],
                                )
                        if probe == "noact":
                            th = stg
                        else:
                            th = tanh_pool.tile(
                                [128, tg * SRC], tanh_dt, tag="tanh", name=f"th{rep}_{b}_{g}"
                            )
                            nc.scalar.activation(th[:, 0 : nb * SRC], stg[:], TANH)
                            for j in range(nb, tg):
                                t = g * tg + j
                                nc.scalar.activation(
                                    th[:, j * SRC : (j + 1) * SRC],
                                    E_sb[:, b, :],
                                    TANH,
                                    bias=D_sb[:, b, t : t + 1],
                                )
                        if probe != "nomm":
                            for j in range(tg):
                                t = g * tg + j
                                for sb in range(NSB):
                                    nc.tensor.matmul(
                                        sc_psum[sb][:, t : t + 1],
                                        th[:, j * SRC + sb * 128 : j * SRC + (sb + 1) * 128],
                                        v_sb[:],
                                        start=True,
                                        stop=True,
                                    )
                    # drain: [s,t] psum -> sbuf -> PE transpose -> [t,s] -> DRAM
                    if probe == "noout":
                        sc_sb0 = scout_pool.tile(
                            [128, 8], f32, tag="scsb", name=f"scsbn{rep}_{b}"
                        )
                        for sb in range(NSB):
                            nc.vector.tensor_copy(
                                sc_sb0[:, sb * 4 : sb * 4 + 4], sc_psum[sb][:, 0:4]
                            )
                        if b == 0:
                            ot = otile_pool.tile(
                                [128, SRC], f32, tag="ot", name=f"otn{rep}"
                            )
                            nc.vector.memset(ot[:], 0.0)
                            nc.sync.dma_start(out=out[0:128, 0, :], in_=ot[:])
                        continue
                    sc_sb = [
                        scout_pool.tile(
                            [128, trg], f32, tag="scsb", name=f"scsb{rep}_{b}_{sb}"
                        )
                        for sb in range(NSB)
                    ]
                    for sb in range(NSB):
                        nc.vector.tensor_copy(sc_sb[sb][:], sc_psum[sb][:])
                    for th_half in range(trg // 128):
                        ot = otile_pool.tile(
                            [128, SRC], f32, tag="ot", name=f"ot{rep}_{b}_{th_half}"
                        )
                        for sb in range(NSB):
                            ps = tp_ps.tile(
                                [128, 128], f32, tag="tp", name=f"tpo{rep}_{b}{th_half}{sb}"
                            )
                            nc.tensor.transpose(
                                ps[:],
                                sc_sb[sb][:, th_half * 128 : (th_half + 1) * 128],
                                ident[:],
                            )
                            nc.vector.tensor_copy(
                                ot[:, sb * 128 : (sb + 1) * 128], ps[:]
                            )
                        nc.sync.dma_start(
                            out=out[th_half * 128 : (th_half + 1) * 128, b, :], in_=ot[:]
                        )
    nc.compile()
    return nc


def _get_nc(trg=TRG):
    key = trg
    if key not in _NC_CACHE:
        _NC_CACHE[key] = build_nc(trg=trg)
    return _NC_CACHE[key]


def _prep_in_maps(inputs):
    dec_out = np.ascontiguousarray(np.asarray(inputs["dec_out"], dtype=np.float32))
    enc_outs = np.ascontiguousarray(np.asarray(inputs["enc_outs"], dtype=np.float32))
    W_s = np.asarray(inputs["W_s"], dtype=np.float32)
    W_t = np.asarray(inputs["W_t"], dtype=np.float32)
    b_t = np.asarray(inputs["b_t"], dtype=np.float32).reshape(ATT, 1)
    v_a = np.asarray(inputs["v_a"], dtype=np.float32).reshape(ATT, 1)

    in_maps = []
    for c in range(N_CORES):
        bsl = slice(c * BC, (c + 1) * BC)
        in_maps.append(
            {
                "dec_out": np.ascontiguousarray(dec_out[:, bsl, :]),
                "enc_outs": np.ascontiguousarray(enc_outs[:, bsl, :]),
                "W_s": W_s,
                "W_t": W_t,
                "b_t": b_t,
                "v_a": v_a,
                "ident128": np.eye(128, dtype=np.float32),
            }
        )
    return in_maps


def kernel(dec_out, enc_outs, W_s, W_t, b_t, v_a):
    from concourse.bass_utils import run_bass_kernel_spmd

    nc = _get_nc()
    in_maps = _prep_in_maps(
        {
            "dec_out": dec_out,
            "enc_outs": enc_outs,
            "W_s": W_s,
            "W_t": W_t,
            "b_t": b_t,
            "v_a": v_a,
        }
    )
    res = run_bass_kernel_spmd(nc, in_maps, list(range(N_CORES)))
    return np.concatenate([r["scores"] for r in res.results], axis=1)



# revision 2
# speedup vs baseline: 1.0326x; 1.0326x over previous
"""Bahdanau scorer via separable rank decomposition (Trainium2, Bass/Tile).

scores[t,b,s] = sum_a v_a * tanh(E[a,s] + D[a,t]),  E = W_s enc^T, D = W_t dec^T + b_t

Key trick: the ACT engine's Sin table is only valid for |arg| <~ 3.8, so
tanh(x+y) is expanded as sum_k C_k sin(w_k(x+y)) = sum_k C_k [sin(w_k x)
cos(w_k y) + cos(w_k x) sin(w_k y)] -- a rank-2R separable form -- with
range reduction done in fast f32 DVE ops via the 1.5*2^23 magic-number
round: m = u - round(u) in [-0.5, 0.5], sin(2 pi u) = sin(2 pi m).

scores[t,s] = sum_{k,a} [C_k v_a cos_k(D)[a,t]] * [sin_k(E)[a,s]] + (swap)
is then a PE matmul with contraction 2R*128, fp16 operands, fp32 PSUM
accumulation, landing directly in [t, s] orientation (no output transpose).
Weights are host-pretransposed to fp16; enc/dec transposed by PE+identity.
"""

import numpy as np

SRC, TRG, BATCH, HID, ATT = 256, 256, 32, 512, 128
N_CORES = 8
BC = BATCH // N_CORES

import os as _os

# Fourier fit of tanh(x) on [-9.3, 9.3]: tanh(x) ~= sum_k C[k] sin(FREQ[k] x),
# max abs fit error 7.2e-3 (10 terms, least-squares with minimax reweighting)
FREQ = np.array([0.28049934407051724, 0.5609986881410345, 0.8414980322115516,
                 1.121997376282069, 1.402496720352586, 1.6829960644231032,
                 1.9634954084936205, 2.243994752564138, 2.5244940966346547,
                 2.804993440705172])
C = np.array([1.23413645589884, -0.001074326085440308, 0.3212195767510474,
              0.004232634204585427, 0.11741316647862238, 0.011299951659352471,
              0.03825121652322847, 0.01347067188190923, 0.00909833660163761,
              0.013468026846218053])

_NC_CACHE = {}


def build_nc():
    import concourse.tile as tile
    from concourse import bacc, mybir

    f32 = mybir.dt.float32
    f16 = mybir.dt.float16
    i32 = mybir.dt.int32
    SIN = mybir.ActivationFunctionType.Sin
    TWO_PI = 6.283185307179586
    A = mybir.AluOpType
    TANH = mybir.ActivationFunctionType.Tanh
    SQUARE = mybir.ActivationFunctionType.Square
    EXP = mybir.ActivationFunctionType.Exp
    R = len(C)
    NHB = HID // 128  # 4
    NTB = TRG // 128  # 2

    nc = bacc.Bacc("TRN2", target_bir_lowering=False, debug=False, num_devices=N_CORES)
    dec_in = nc.dram_tensor("dec_out", [TRG, BC, HID], f32, kind="ExternalInput")
    enc_in = nc.dram_tensor("enc_outs", [SRC, BC, HID], f32, kind="ExternalInput")
    id_in = nc.dram_tensor("ident128", [128, 128], f32, kind="ExternalInput")
    wsT_in = nc.dram_tensor("wsT16", [HID, ATT], f16, kind="ExternalInput")
    wtT_in = nc.dram_tensor("wtT16", [HID, ATT], f16, kind="ExternalInput")
    bt_in = nc.dram_tensor("b_t", [ATT, 1], f32, kind="ExternalInput")
    vs_in = nc.dram_tensor("vs", [ATT, R], f32, kind="ExternalInput")
    bE_in = nc.dram_tensor("betasE", [128, R], f32, kind="ExternalInput")
    bD_in = nc.dram_tensor("betasD", [128, R], f32, kind="ExternalInput")
    out = nc.dram_tensor("scores", [TRG, BC, SRC], f32, kind="ExternalOutput")


    with tile.TileContext(nc) as tc:
        with (
            tc.tile_pool(name="consts", bufs=1) as consts,
            tc.tile_pool(name="raw", bufs=4) as rawp,
            tc.tile_pool(name="xt", bufs=3) as xtp,
            tc.tile_pool(name="ed", bufs=1) as ed,
            tc.tile_pool(name="fE", bufs=5) as fEp,
            tc.tile_pool(name="fD", bufs=5) as fDp,
            tc.tile_pool(name="mid", bufs=5) as midp,
            tc.tile_pool(name="oden", bufs=3) as odenp,
            tc.tile_pool(name="ps", bufs=8, space="PSUM") as ps_pool,
        ):
            halfpi = consts.tile([128, 1], f32)
            nc.vector.memset(halfpi[:], 1.5707963267948966)
            ident = consts.tile([128, 128], f32)
            nc.sync.dma_start(out=ident[:], in_=id_in[:])
            # consts + warm the ACT table set off the critical path
            warm = consts.tile([1, 2], f32)
            nc.vector.memset(warm[:], 0.0)
            nc.scalar.activation(warm[:], warm[:], TANH)
            nc.scalar.activation(warm[:1], warm[:1], SIN)
            vs_sb = consts.tile([128, R], f32)
            nc.sync.dma_start(out=vs_sb[:], in_=vs_in[:])
            bE_sb = consts.tile([128, R], f32)
            nc.sync.dma_start(out=bE_sb[:], in_=bE_in[:])
            bD_sb = consts.tile([128, R], f32)
            nc.sync.dma_start(out=bD_sb[:], in_=bD_in[:])
            bt_sb = consts.tile([128, 1], f32)
            nc.sync.dma_start(out=bt_sb[:], in_=bt_in[:])
            wT = {}
            for name, w_in in (("s", wsT_in), ("t", wtT_in)):
                wT[name] = consts.tile([128, NHB, 128], f16, name=f"w{name}T")
                for hb in range(NHB):
                    nc.sync.dma_start(
                        out=wT[name][:, hb, :], in_=w_in[hb * 128 : (hb + 1) * 128, :]
                    )

            # --- staging: ED_sb [128a, (E: BC*256 | D: BC*256)] fp32 ---
            ED_sb = ed.tile([128, 2, BC, SRC], f32, name="ED")
            for b in range(BC):
                for name, src_dram, half in (("enc", enc_in, 0), ("dec", dec_in, 1)):
                    xT = xtp.tile([128, NHB, 256], f16, tag="xT", name=f"xT_{b}{name}")
                    for fb in range(2):
                        x_raw = rawp.tile(
                            [128, HID], f32, tag="raw", name=f"raw_{b}{name}{fb}"
                        )
                        nc.sync.dma_start(
                            out=x_raw[:], in_=src_dram[fb * 128 : (fb + 1) * 128, b, :]
                        )
                        for hb in range(NHB):
                            tp = ps_pool.tile(
                                [128, 512], f32, tag="ps", name=f"tp_{b}{name}{fb}{hb}"
                            )
                            nc.tensor.transpose(
                                tp[:, 0:128], x_raw[:, hb * 128 : (hb + 1) * 128], ident[:]
                            )
                            nc.vector.tensor_copy(
                                xT[:, hb, fb * 128 : (fb + 1) * 128], tp[:, 0:128]
                            )
                    w_key = "s" if name == "enc" else "t"
                    pps = ps_pool.tile([128, 512], f32, tag="ps", name=f"pj_{b}{name}")
                    for hb in range(NHB):
                        nc.tensor.matmul(
                            pps[:, 0:256],
                            wT[w_key][:, hb, :],
                            xT[:, hb, :],
                            start=(hb == 0),
                            stop=(hb == NHB - 1),
                        )
                    if name == "enc":
                        nc.vector.tensor_copy(ED_sb[:, half, b, :], pps[:, 0:256])
                    else:
                        nc.vector.tensor_scalar_add(
                            ED_sb[:, half, b, :], pps[:, 0:256], bt_sb[:, 0:1]
                        )

            E_view = ED_sb[:, 0]  # [128, BC, 256]
            D_view = ED_sb[:, 1]

            # --- main loop over components ---
            # one full PSUM bank per (b, tb) region: start=True clears
            # has_written bank-wide, so accumulating regions must not share
            sc_psum = [
                [
                    ps_pool.tile([128, 512], f32, tag="ps", name=f"sc_{b}_{tb}")
                    for tb in range(NTB)
                ]
                for b in range(BC)
            ]
            # --- fourier main loop: R freqs -> 2R components ---
            EDW = 2 * BC * SRC
            for k in range(R):
                ck = float(FREQ[k] / TWO_PI)
                MAGIC = 12582912.0  # 1.5*2^23: fp32 add/sub rounds to nearest int
                # u = c*ED; r1 = round(u); m1 = u - r1  (all fast f32 DVE ops)
                u1 = midp.tile([128, EDW], f32, tag="mf", name=f"u1_{k}")
                nc.vector.tensor_scalar(u1[:], ED_sb[:], ck, None, A.mult)
                r1 = midp.tile([128, EDW], f32, tag="mf", name=f"r1_{k}")
                nc.vector.tensor_scalar(r1[:], u1[:], MAGIC, MAGIC, A.add, A.subtract)
                m1 = midp.tile([128, EDW], f32, tag="mf", name=f"m1_{k}")
                nc.vector.tensor_tensor(m1[:], u1[:], r1[:], A.subtract)
                sinf = fEp.tile([128, EDW], f16, tag="fE", name=f"sin{k}")
                nc.scalar.activation(sinf[:], m1[:], SIN, scale=TWO_PI)
                # r2 = round(u + 0.25); m2 = u - r2;  cos(2pi*u) = sin(2pi*m2 + pi/2)
                r2 = midp.tile([128, EDW], f32, tag="mf", name=f"r2_{k}")
                nc.vector.tensor_scalar(r2[:], u1[:], MAGIC + 0.25, MAGIC, A.add, A.subtract)
                m2 = midp.tile([128, EDW], f32, tag="mf", name=f"m2_{k}")
                nc.vector.tensor_tensor(m2[:], u1[:], r2[:], A.subtract)
                cosf = fEp.tile([128, EDW], f16, tag="fE", name=f"cos{k}")
                nc.scalar.activation(cosf[:], m2[:], SIN, bias=halfpi[:, 0:1], scale=TWO_PI)
                # D-side lhsT tiles scaled by a_k*v_a
                fDA = fDp.tile([128, BC * TRG], f16, tag="fD", name=f"fDA{k}")
                nc.vector.tensor_scalar_mul(fDA[:], cosf[:, BC * SRC :], vs_sb[:, k : k + 1])
                fDB = fDp.tile([128, BC * TRG], f16, tag="fD", name=f"fDB{k}")
                nc.vector.tensor_scalar_mul(fDB[:], sinf[:, BC * SRC :], vs_sb[:, k : k + 1])
                for b in range(BC):
                    for tb in range(NTB):
                        nc.tensor.matmul(
                            sc_psum[b][tb][:, 0:256],
                            fDA[:, b * TRG + tb * 128 : b * TRG + (tb + 1) * 128],
                            sinf[:, b * SRC : (b + 1) * SRC],
                            start=(k == 0),
                            stop=False,
                        )
                        nc.tensor.matmul(
                            sc_psum[b][tb][:, 0:256],
                            fDB[:, b * TRG + tb * 128 : b * TRG + (tb + 1) * 128],
                            cosf[:, b * SRC : (b + 1) * SRC],
                            start=False,
                            stop=(k == R - 1),
                        )

            # --- drain ---
            for b in range(BC):
                for tb in range(NTB):
                    ot = odenp.tile([128, SRC], f32, tag="ot", name=f"ot_{b}_{tb}")
                    nc.vector.tensor_copy(ot[:], sc_psum[b][tb][:, 0:256])
                    nc.sync.dma_start(out=out[tb * 128 : (tb + 1) * 128, b, :], in_=ot[:])
    nc.compile()
    return nc


def _get_nc():
    if "nc" not in _NC_CACHE:
        _NC_CACHE["nc"] = build_nc()
    return _NC_CACHE["nc"]


def _prep_in_maps(inputs):
    dec_out = np.ascontiguousarray(np.asarray(inputs["dec_out"], np.float32))
    enc_outs = np.ascontiguousarray(np.asarray(inputs["enc_outs"], np.float32))
    wsT16 = np.ascontiguousarray(np.asarray(inputs["W_s"], np.float32).T.astype(np.float16))
    wtT16 = np.ascontiguousarray(np.asarray(inputs["W_t"], np.float32).T.astype(np.float16))
    b_t = np.asarray(inputs["b_t"], np.float32).reshape(ATT, 1)
    v_a = np.asarray(inputs["v_a"], np.float32).reshape(ATT, 1)

    R = len(C)
    vs = (v_a[:, 0:1] * np.asarray(C, np.float32)[None, :]).astype(np.float32)
    betasE = np.zeros((128, R), np.float32)
    betasD = np.zeros((128, R), np.float32)

    in_maps = []
    for c in range(N_CORES):
        bsl = slice(c * BC, (c + 1) * BC)
        in_maps.append(
            {
                "dec_out": np.ascontiguousarray(dec_out[:, bsl, :]),
                "enc_outs": np.ascontiguousarray(enc_outs[:, bsl, :]),
                "ident128": np.eye(128, dtype=np.float32),
                "wsT16": wsT16,
                "wtT16": wtT16,
                "b_t": b_t,
                "vs": vs,
                "betasE": betasE,
                "betasD": betasD,
            }
        )
    return in_maps


def kernel(dec_out, enc_outs, W_s, W_t, b_t, v_a):
    from concourse.bass_utils import run_bass_kernel_spmd

    nc = _get_nc()
    in_maps = _prep_in_maps(
        {
            "dec_out": dec_out,
            "enc_outs": enc_outs,
            "W_s": W_s,
            "W_t": W_t,
            "b_t": b_t,
            "v_a": v_a,
        }
    )
    res = run_bass_kernel_spmd(nc, in_maps, list(range(N_CORES)))
    return np.concatenate([r["scores"] for r in res.results], axis=1)


# revision 3
# speedup vs baseline: 1.0816x; 1.0474x over previous
"""Bahdanau scorer via separable rank decomposition (Trainium2, Bass/Tile).

scores[t,b,s] = sum_a v_a * tanh(E[a,s] + D[a,t]),  E = W_s enc^T, D = W_t dec^T + b_t

Key trick: the ACT engine's Sin table is only valid for |arg| <~ 3.8, so
tanh(x+y) is expanded as sum_k C_k sin(w_k(x+y)) = sum_k C_k [sin(w_k x)
cos(w_k y) + cos(w_k x) sin(w_k y)] -- a rank-2R separable form -- with
range reduction done in fast f32 DVE ops via the 1.5*2^23 magic-number
round: m = u - round(u) in [-0.5, 0.5], sin(2 pi u) = sin(2 pi m).

scores[t,s] = sum_{k,a} [C_k v_a cos_k(D)[a,t]] * [sin_k(E)[a,s]] + (swap)
is then a PE matmul with contraction 2R*128, fp16 operands, fp32 PSUM
accumulation, landing directly in [t, s] orientation (no output transpose).
Weights are host-pretransposed to fp16; enc/dec transposed by PE+identity.
"""

import numpy as np

SRC, TRG, BATCH, HID, ATT = 256, 256, 32, 512, 128
N_CORES = 8
BC = BATCH // N_CORES

import os as _os

# Fourier fit of tanh(x) on [-9.3, 9.3]: tanh(x) ~= sum_k C[k] sin(FREQ[k] x),
# max abs fit error 7.2e-3 (10 terms, least-squares with minimax reweighting)
FREQ = np.array([0.28049934407051724, 0.5609986881410345, 0.8414980322115516,
                 1.121997376282069, 1.402496720352586, 1.6829960644231032,
                 1.9634954084936205, 2.243994752564138, 2.5244940966346547,
                 2.804993440705172])
C = np.array([1.23413645589884, -0.001074326085440308, 0.3212195767510474,
              0.004232634204585427, 0.11741316647862238, 0.011299951659352471,
              0.03825121652322847, 0.01347067188190923, 0.00909833660163761,
              0.013468026846218053])

_NC_CACHE = {}


def build_nc():
    import concourse.tile as tile
    from concourse import bacc, mybir

    f32 = mybir.dt.float32
    f16 = mybir.dt.float16
    i32 = mybir.dt.int32
    SIN = mybir.ActivationFunctionType.Sin
    TWO_PI = 6.283185307179586
    A = mybir.AluOpType
    TANH = mybir.ActivationFunctionType.Tanh
    SQUARE = mybir.ActivationFunctionType.Square
    EXP = mybir.ActivationFunctionType.Exp
    R = len(C)
    NHB = HID // 128  # 4
    NTB = TRG // 128  # 2

    nc = bacc.Bacc("TRN2", target_bir_lowering=False, debug=False, num_devices=N_CORES)
    dec_in = nc.dram_tensor("dec_out", [TRG, BC, HID], f32, kind="ExternalInput")
    enc_in = nc.dram_tensor("enc_outs", [SRC, BC, HID], f32, kind="ExternalInput")
    id_in = nc.dram_tensor("ident128", [128, 128], f32, kind="ExternalInput")
    wsT_in = nc.dram_tensor("wsT16", [HID, ATT], f16, kind="ExternalInput")
    wtT_in = nc.dram_tensor("wtT16", [HID, ATT], f16, kind="ExternalInput")
    bt_in = nc.dram_tensor("b_t", [ATT, 1], f32, kind="ExternalInput")
    vs_in = nc.dram_tensor("vs", [ATT, R], f32, kind="ExternalInput")
    bE_in = nc.dram_tensor("betasE", [128, R], f32, kind="ExternalInput")
    bD_in = nc.dram_tensor("betasD", [128, R], f32, kind="ExternalInput")
    out = nc.dram_tensor("scores", [TRG, BC, SRC], f32, kind="ExternalOutput")


    with tile.TileContext(nc) as tc:
        with (
            tc.tile_pool(name="consts", bufs=1) as consts,
            tc.tile_pool(name="raw", bufs=4) as rawp,
            tc.tile_pool(name="xt", bufs=3) as xtp,
            tc.tile_pool(name="ed", bufs=1) as ed,
            tc.tile_pool(name="fE", bufs=5) as fEp,
            tc.tile_pool(name="fD", bufs=5) as fDp,
            tc.tile_pool(name="mid", bufs=5) as midp,
            tc.tile_pool(name="oden", bufs=3) as odenp,
            tc.tile_pool(name="ps", bufs=8, space="PSUM") as ps_pool,
        ):
            halfpi = consts.tile([128, 1], f32)
            nc.vector.memset(halfpi[:], 1.5707963267948966)
            ident = consts.tile([128, 128], f32)
            nc.sync.dma_start(out=ident[:], in_=id_in[:])
            # consts + warm the ACT table set off the critical path
            warm = consts.tile([1, 2], f32)
            nc.vector.memset(warm[:], 0.0)
            nc.scalar.activation(warm[:], warm[:], TANH)
            nc.scalar.activation(warm[:1], warm[:1], SIN)
            vs_sb = consts.tile([128, R], f32)
            nc.sync.dma_start(out=vs_sb[:], in_=vs_in[:])
            bE_sb = consts.tile([128, R], f32)
            nc.sync.dma_start(out=bE_sb[:], in_=bE_in[:])
            bD_sb = consts.tile([128, R], f32)
            nc.sync.dma_start(out=bD_sb[:], in_=bD_in[:])
            bt_sb = consts.tile([128, 1], f32)
            nc.sync.dma_start(out=bt_sb[:], in_=bt_in[:])
            wT = {}
            for name, w_in in (("s", wsT_in), ("t", wtT_in)):
                wT[name] = consts.tile([128, NHB, 128], f16, name=f"w{name}T")
                for hb in range(NHB):
                    nc.sync.dma_start(
                        out=wT[name][:, hb, :], in_=w_in[hb * 128 : (hb + 1) * 128, :]
                    )

            # --- staging: ED_sb [128a, (E: BC*256 | D: BC*256)] fp32 ---
            ED_sb = ed.tile([128, 2, BC, SRC], f32, name="ED")
            for b in range(BC):
                for name, src_dram, half in (("enc", enc_in, 0), ("dec", dec_in, 1)):
                    xT = xtp.tile([128, NHB, 256], f16, tag="xT", name=f"xT_{b}{name}")
                    for fb in range(2):
                        x_raw = rawp.tile(
                            [128, HID], f32, tag="raw", name=f"raw_{b}{name}{fb}"
                        )
                        nc.sync.dma_start(
                            out=x_raw[:], in_=src_dram[fb * 128 : (fb + 1) * 128, b, :]
                        )
                        for hb in range(NHB):
                            tp = ps_pool.tile(
                                [128, 512], f32, tag="ps", name=f"tp_{b}{name}{fb}{hb}"
                            )
                            nc.tensor.transpose(
                                tp[:, 0:128], x_raw[:, hb * 128 : (hb + 1) * 128], ident[:]
                            )
                            if hb % 2 == 0:
                                nc.vector.tensor_copy(
                                    xT[:, hb, fb * 128 : (fb + 1) * 128], tp[:, 0:128]
                                )
                            else:
                                nc.scalar.copy(
                                    xT[:, hb, fb * 128 : (fb + 1) * 128], tp[:, 0:128]
                                )
                    w_key = "s" if name == "enc" else "t"
                    pps = ps_pool.tile([128, 512], f32, tag="ps", name=f"pj_{b}{name}")
                    for hb in range(NHB):
                        nc.tensor.matmul(
                            pps[:, 0:256],
                            wT[w_key][:, hb, :],
                            xT[:, hb, :],
                            start=(hb == 0),
                            stop=(hb == NHB - 1),
                        )
                    if name == "enc":
                        nc.vector.tensor_copy(ED_sb[:, half, b, :], pps[:, 0:256])
                    else:
                        nc.vector.tensor_scalar_add(
                            ED_sb[:, half, b, :], pps[:, 0:256], bt_sb[:, 0:1]
                        )

            E_view = ED_sb[:, 0]  # [128, BC, 256]
            D_view = ED_sb[:, 1]

            # --- main loop over components ---
            # one full PSUM bank per (b, tb) region: start=True clears
            # has_written bank-wide, so accumulating regions must not share
            sc_psum = [
                [
                    ps_pool.tile([128, 512], f32, tag="ps", name=f"sc_{b}_{tb}")
                    for tb in range(NTB)
                ]
                for b in range(BC)
            ]
            # --- fourier main loop: R freqs -> 2R components ---
            EDW = 2 * BC * SRC
            for k in range(R):
                ck = float(FREQ[k] / TWO_PI)
                MAGIC = 12582912.0  # 1.5*2^23: fp32 add/sub rounds to nearest int
                # u = c*ED; r1 = round(u); m1 = u - r1  (all fast f32 DVE ops)
                u1 = midp.tile([128, EDW], f32, tag="mf", name=f"u1_{k}")
                nc.scalar.mul(u1[:], ED_sb[:], ck)
                r1 = midp.tile([128, EDW], f32, tag="mf", name=f"r1_{k}")
                nc.vector.tensor_scalar(r1[:], u1[:], MAGIC, MAGIC, A.add, A.subtract)
                m1 = midp.tile([128, EDW], f32, tag="mf", name=f"m1_{k}")
                nc.vector.tensor_tensor(m1[:], u1[:], r1[:], A.subtract)
                sinf = fEp.tile([128, EDW], f16, tag="fE", name=f"sin{k}")
                nc.scalar.activation(sinf[:], m1[:], SIN, scale=TWO_PI)
                # r2 = round(u + 0.25); m2 = u - r2;  cos(2pi*u) = sin(2pi*m2 + pi/2)
                r2 = midp.tile([128, EDW], f32, tag="mf", name=f"r2_{k}")
                nc.vector.tensor_scalar(r2[:], u1[:], MAGIC + 0.25, MAGIC, A.add, A.subtract)
                m2 = midp.tile([128, EDW], f32, tag="mf", name=f"m2_{k}")
                nc.vector.tensor_tensor(m2[:], u1[:], r2[:], A.subtract)
                cosf = fEp.tile([128, EDW], f16, tag="fE", name=f"cos{k}")
                nc.scalar.activation(cosf[:], m2[:], SIN, bias=halfpi[:, 0:1], scale=TWO_PI)
                # D-side lhsT tiles scaled by a_k*v_a
                fDA = fDp.tile([128, BC * TRG], f16, tag="fD", name=f"fDA{k}")
                nc.vector.tensor_scalar_mul(fDA[:], cosf[:, BC * SRC :], vs_sb[:, k : k + 1])
                fDB = fDp.tile([128, BC * TRG], f16, tag="fD", name=f"fDB{k}")
                nc.vector.tensor_scalar_mul(fDB[:], sinf[:, BC * SRC :], vs_sb[:, k : k + 1])
                for b in range(BC):
                    for tb in range(NTB):
                        nc.tensor.matmul(
                            sc_psum[b][tb][:, 0:256],
                            fDA[:, b * TRG + tb * 128 : b * TRG + (tb + 1) * 128],
                            sinf[:, b * SRC : (b + 1) * SRC],
                            start=(k == 0),
                            stop=False,
                        )
                        nc.tensor.matmul(
                            sc_psum[b][tb][:, 0:256],
                            fDB[:, b * TRG + tb * 128 : b * TRG + (tb + 1) * 128],
                            cosf[:, b * SRC : (b + 1) * SRC],
                            start=False,
                            stop=(k == R - 1),
                        )

            # --- drain ---
            for b in range(BC):
                for tb in range(NTB):
                    ot = odenp.tile([128, SRC], f32, tag="ot", name=f"ot_{b}_{tb}")
                    nc.vector.tensor_copy(ot[:], sc_psum[b][tb][:, 0:256])
                    nc.sync.dma_start(out=out[tb * 128 : (tb + 1) * 128, b, :], in_=ot[:])
    nc.compile()
    return nc


def _get_nc():
    if "nc" not in _NC_CACHE:
        _NC_CACHE["nc"] = build_nc()
    return _NC_CACHE["nc"]


def _prep_in_maps(inputs):
    dec_out = np.ascontiguousarray(np.asarray(inputs["dec_out"], np.float32))
    enc_outs = np.ascontiguousarray(np.asarray(inputs["enc_outs"], np.float32))
    wsT16 = np.ascontiguousarray(np.asarray(inputs["W_s"], np.float32).T.astype(np.float16))
    wtT16 = np.ascontiguousarray(np.asarray(inputs["W_t"], np.float32).T.astype(np.float16))
    b_t = np.asarray(inputs["b_t"], np.float32).reshape(ATT, 1)
    v_a = np.asarray(inputs["v_a"], np.float32).reshape(ATT, 1)

    R = len(C)
    vs = (v_a[:, 0:1] * np.asarray(C, np.float32)[None, :]).astype(np.float32)
    betasE = np.zeros((128, R), np.float32)
    betasD = np.zeros((128, R), np.float32)

    in_maps = []
    for c in range(N_CORES):
        bsl = slice(c * BC, (c + 1) * BC)
        in_maps.append(
            {
                "dec_out": np.ascontiguousarray(dec_out[:, bsl, :]),
                "enc_outs": np.ascontiguousarray(enc_outs[:, bsl, :]),
                "ident128": np.eye(128, dtype=np.float32),
                "wsT16": wsT16,
                "wtT16": wtT16,
                "b_t": b_t,
                "vs": vs,
                "betasE": betasE,
                "betasD": betasD,
            }
        )
    return in_maps


def kernel(dec_out, enc_outs, W_s, W_t, b_t, v_a):
    from concourse.bass_utils import run_bass_kernel_spmd

    nc = _get_nc()
    in_maps = _prep_in_maps(
        {
            "dec_out": dec_out,
            "enc_outs": enc_outs,
            "W_s": W_s,
            "W_t": W_t,
            "b_t": b_t,
            "v_a": v_a,
        }
    )
    res = run_bass_kernel_spmd(nc, in_maps, list(range(N_CORES)))
    return np.concatenate([r["scores"] for r in res.results], axis=1)


# revision 5
# speedup vs baseline: 1.1201x; 1.0357x over previous
"""Bahdanau scorer via separable rank decomposition (Trainium2, Bass/Tile).

scores[t,b,s] = sum_a v_a * tanh(E[a,s] + D[a,t]),  E = W_s enc^T, D = W_t dec^T + b_t

Key trick: the ACT engine's Sin table is only valid for |arg| <~ 3.8, so
tanh(x+y) is expanded as sum_k C_k sin(w_k(x+y)) = sum_k C_k [sin(w_k x)
cos(w_k y) + cos(w_k x) sin(w_k y)] -- a rank-2R separable form -- with
range reduction done in fast f32 DVE ops via the 1.5*2^23 magic-number
round: m = u - round(u) in [-0.5, 0.5], sin(2 pi u) = sin(2 pi m).

scores[t,s] = sum_{k,a} [C_k v_a cos_k(D)[a,t]] * [sin_k(E)[a,s]] + (swap)
is then a PE matmul with contraction 2R*128, fp16 operands, fp32 PSUM
accumulation, landing directly in [t, s] orientation (no output transpose).
Weights are host-pretransposed to fp16; enc/dec transposed by PE+identity.
"""

import numpy as np

SRC, TRG, BATCH, HID, ATT = 256, 256, 32, 512, 128
N_CORES = 8
BC = BATCH // N_CORES

import os as _os

# Fourier fit of tanh(x) on [-9.3, 9.3]: tanh(x) ~= sum_k C[k] sin(FREQ[k] x),
# max abs fit error 7.2e-3 (10 terms, least-squares with minimax reweighting)
FREQ = np.array([0.28049934407051724, 0.5609986881410345, 0.8414980322115516,
                 1.121997376282069, 1.402496720352586, 1.6829960644231032,
                 1.9634954084936205, 2.243994752564138, 2.5244940966346547,
                 2.804993440705172])
C = np.array([1.23413645589884, -0.001074326085440308, 0.3212195767510474,
              0.004232634204585427, 0.11741316647862238, 0.011299951659352471,
              0.03825121652322847, 0.01347067188190923, 0.00909833660163761,
              0.013468026846218053])

_NC_CACHE = {}


def build_nc():
    import concourse.tile as tile
    from concourse import bacc, mybir

    f32 = mybir.dt.float32
    f16 = mybir.dt.float16
    i32 = mybir.dt.int32
    SIN = mybir.ActivationFunctionType.Sin
    TWO_PI = 6.283185307179586
    A = mybir.AluOpType
    TANH = mybir.ActivationFunctionType.Tanh
    SQUARE = mybir.ActivationFunctionType.Square
    EXP = mybir.ActivationFunctionType.Exp
    R = len(C)
    NHB = HID // 128  # 4
    NTB = TRG // 128  # 2

    nc = bacc.Bacc("TRN2", target_bir_lowering=False, debug=False, num_devices=N_CORES)
    dec_in = nc.dram_tensor("dec_out", [TRG, BC, HID], f32, kind="ExternalInput")
    enc_in = nc.dram_tensor("enc_outs", [SRC, BC, HID], f32, kind="ExternalInput")
    id_in = nc.dram_tensor("ident128", [128, 128], f32, kind="ExternalInput")
    wsT_in = nc.dram_tensor("wsT16", [HID, ATT], f16, kind="ExternalInput")
    wtT_in = nc.dram_tensor("wtT16", [HID, ATT], f16, kind="ExternalInput")
    bt_in = nc.dram_tensor("b_t", [ATT, 1], f32, kind="ExternalInput")
    vs_in = nc.dram_tensor("vs", [ATT, R], f32, kind="ExternalInput")
    bE_in = nc.dram_tensor("betasE", [128, R], f32, kind="ExternalInput")
    bD_in = nc.dram_tensor("betasD", [128, R], f32, kind="ExternalInput")
    out = nc.dram_tensor("scores", [TRG, BC, SRC], f32, kind="ExternalOutput")


    with tile.TileContext(nc) as tc:
        with (
            tc.tile_pool(name="consts", bufs=1) as consts,
            tc.tile_pool(name="raw", bufs=4) as rawp,
            tc.tile_pool(name="xt", bufs=3) as xtp,
            tc.tile_pool(name="ed", bufs=1) as ed,
            tc.tile_pool(name="fE", bufs=5) as fEp,
            tc.tile_pool(name="fD", bufs=5) as fDp,
            tc.tile_pool(name="mid", bufs=5) as midp,
            tc.tile_pool(name="oden", bufs=3) as odenp,
            tc.tile_pool(name="ps", bufs=8, space="PSUM") as ps_pool,
        ):
            halfpi = consts.tile([128, 1], f32)
            nc.vector.memset(halfpi[:], 1.5707963267948966)
            ident = consts.tile([128, 128], f32)
            nc.sync.dma_start(out=ident[:], in_=id_in[:])
            # consts + warm the ACT table set off the critical path
            warm = consts.tile([1, 2], f32)
            nc.vector.memset(warm[:], 0.0)
            nc.scalar.activation(warm[:], warm[:], TANH)
            nc.scalar.activation(warm[:1], warm[:1], SIN)
            vs_sb = consts.tile([128, R], f32)
            nc.sync.dma_start(out=vs_sb[:], in_=vs_in[:])
            bE_sb = consts.tile([128, R], f32)
            nc.sync.dma_start(out=bE_sb[:], in_=bE_in[:])
            bD_sb = consts.tile([128, R], f32)
            nc.sync.dma_start(out=bD_sb[:], in_=bD_in[:])
            bt_sb = consts.tile([128, 1], f32)
            nc.sync.dma_start(out=bt_sb[:], in_=bt_in[:])
            wT = {}
            for name, w_in in (("s", wsT_in), ("t", wtT_in)):
                wT[name] = consts.tile([128, NHB, 128], f16, name=f"w{name}T")
                for hb in range(NHB):
                    nc.sync.dma_start(
                        out=wT[name][:, hb, :], in_=w_in[hb * 128 : (hb + 1) * 128, :]
                    )

            # --- staging: ED_sb [128a, (E: BC*256 | D: BC*256)] fp32 ---
            ED_sb = ed.tile([128, 2, BC, SRC], f32, name="ED")
            for b in range(BC):
                for name, src_dram, half in (("enc", enc_in, 0), ("dec", dec_in, 1)):
                    xT = xtp.tile([128, NHB, 256], f16, tag="xT", name=f"xT_{b}{name}")
                    for fb in range(2):
                        x_raw = rawp.tile(
                            [128, HID], f32, tag="raw", name=f"raw_{b}{name}{fb}"
                        )
                        nc.sync.dma_start(
                            out=x_raw[:], in_=src_dram[fb * 128 : (fb + 1) * 128, b, :]
                        )
                        for hb in range(NHB):
                            tp = ps_pool.tile(
                                [128, 512], f32, tag="ps", name=f"tp_{b}{name}{fb}{hb}"
                            )
                            nc.tensor.transpose(
                                tp[:, 0:128], x_raw[:, hb * 128 : (hb + 1) * 128], ident[:]
                            )
                            if hb % 2 == 0:
                                nc.vector.tensor_copy(
                                    xT[:, hb, fb * 128 : (fb + 1) * 128], tp[:, 0:128]
                                )
                            else:
                                nc.scalar.copy(
                                    xT[:, hb, fb * 128 : (fb + 1) * 128], tp[:, 0:128]
                                )
                    w_key = "s" if name == "enc" else "t"
                    pps = ps_pool.tile([128, 512], f32, tag="ps", name=f"pj_{b}{name}")
                    for hb in range(NHB):
                        nc.tensor.matmul(
                            pps[:, 0:256],
                            wT[w_key][:, hb, :],
                            xT[:, hb, :],
                            start=(hb == 0),
                            stop=(hb == NHB - 1),
                        )
                    if name == "enc":
                        nc.vector.tensor_copy(ED_sb[:, half, b, :], pps[:, 0:256])
                    else:
                        nc.vector.tensor_scalar_add(
                            ED_sb[:, half, b, :], pps[:, 0:256], bt_sb[:, 0:1]
                        )

            E_view = ED_sb[:, 0]  # [128, BC, 256]
            D_view = ED_sb[:, 1]

            # --- main loop over components ---
            # one full PSUM bank per (b, tb) region: start=True clears
            # has_written bank-wide, so accumulating regions must not share
            sc_psum = [
                [
                    ps_pool.tile([128, 512], f32, tag="ps", name=f"sc_{b}_{tb}")
                    for tb in range(NTB)
                ]
                for b in range(BC)
            ]
            # --- fourier main loop: R freqs -> 2R components ---
            EDW = 2 * BC * SRC
            for k in range(R):
                ck = float(FREQ[k] / TWO_PI)
                MAGIC = 12582912.0  # 1.5*2^23: fp32 add/sub rounds to nearest int
                # u = c*ED; r1 = round(u); m1 = u - r1  (all fast f32 DVE ops)
                u1 = midp.tile([128, EDW], f32, tag="mf", name=f"u1_{k}")
                nc.scalar.mul(u1[:], ED_sb[:], ck)
                r1 = midp.tile([128, EDW], f32, tag="mf", name=f"r1_{k}")
                nc.vector.tensor_scalar(r1[:], u1[:], MAGIC, MAGIC, A.add, A.subtract)
                m1 = midp.tile([128, EDW], f32, tag="mf", name=f"m1_{k}")
                nc.vector.tensor_tensor(m1[:], u1[:], r1[:], A.subtract)
                sinf = fEp.tile([128, EDW], f16, tag="fE", name=f"sin{k}")
                nc.scalar.activation(sinf[:], m1[:], SIN, scale=TWO_PI)
                # r2 = round(u + 0.25); m2 = u - r2;  cos(2pi*u) = sin(2pi*m2 + pi/2)
                r2 = midp.tile([128, EDW], f32, tag="mf", name=f"r2_{k}")
                nc.vector.tensor_scalar(r2[:], u1[:], MAGIC + 0.25, MAGIC, A.add, A.subtract)
                m2 = midp.tile([128, EDW], f32, tag="mf", name=f"m2_{k}")
                nc.vector.tensor_tensor(m2[:], u1[:], r2[:], A.subtract)
                cosf = fEp.tile([128, EDW], f16, tag="fE", name=f"cos{k}")
                nc.scalar.activation(cosf[:], m2[:], SIN, bias=halfpi[:, 0:1], scale=TWO_PI)
                # D-side lhsT tiles scaled by a_k*v_a
                fDA = fDp.tile([128, BC * TRG], f16, tag="fD", name=f"fDA{k}")
                nc.vector.tensor_scalar_mul(fDA[:], cosf[:, BC * SRC :], vs_sb[:, k : k + 1])
                fDB = fDp.tile([128, BC * TRG], f16, tag="fD", name=f"fDB{k}")
                nc.vector.tensor_scalar_mul(fDB[:], sinf[:, BC * SRC :], vs_sb[:, k : k + 1])
                for b in range(BC):
                    for tb in range(NTB):
                        nc.tensor.matmul(
                            sc_psum[b][tb][:, 0:256],
                            fDA[:, b * TRG + tb * 128 : b * TRG + (tb + 1) * 128],
                            sinf[:, b * SRC : (b + 1) * SRC],
                            start=(k == 0),
                            stop=False,
                        )
                        nc.tensor.matmul(
                            sc_psum[b][tb][:, 0:256],
                            fDB[:, b * TRG + tb * 128 : b * TRG + (tb + 1) * 128],
                            cosf[:, b * SRC : (b + 1) * SRC],
                            start=False,
                            stop=(k == R - 1),
                        )

            # --- drain ---
            for b in range(BC):
                for tb in range(NTB):
                    ot = odenp.tile([128, SRC], f32, tag="ot", name=f"ot_{b}_{tb}")
                    nc.vector.tensor_copy(ot[:], sc_psum[b][tb][:, 0:256])
                    nc.sync.dma_start(out=out[tb * 128 : (tb + 1) * 128, b, :], in_=ot[:])
    nc.compile()
    return nc


def _get_nc():
    if "nc" not in _NC_CACHE:
        _NC_CACHE["nc"] = build_nc()
    return _NC_CACHE["nc"]


def _prep_in_maps(inputs):
    dec_out = np.ascontiguousarray(np.asarray(inputs["dec_out"], np.float32))
    enc_outs = np.ascontiguousarray(np.asarray(inputs["enc_outs"], np.float32))
    wsT16 = np.ascontiguousarray(np.asarray(inputs["W_s"], np.float32).T.astype(np.float16))
    wtT16 = np.ascontiguousarray(np.asarray(inputs["W_t"], np.float32).T.astype(np.float16))
    b_t = np.asarray(inputs["b_t"], np.float32).reshape(ATT, 1)
    v_a = np.asarray(inputs["v_a"], np.float32).reshape(ATT, 1)

    R = len(C)
    vs = (v_a[:, 0:1] * np.asarray(C, np.float32)[None, :]).astype(np.float32)
    betasE = np.zeros((128, R), np.float32)
    betasD = np.zeros((128, R), np.float32)

    in_maps = []
    for c in range(N_CORES):
        bsl = slice(c * BC, (c + 1) * BC)
        in_maps.append(
            {
                "dec_out": np.ascontiguousarray(dec_out[:, bsl, :]),
                "enc_outs": np.ascontiguousarray(enc_outs[:, bsl, :]),
                "ident128": np.eye(128, dtype=np.float32),
                "wsT16": wsT16,
                "wtT16": wtT16,
                "b_t": b_t,
                "vs": vs,
                "betasE": betasE,
                "betasD": betasD,
            }
        )
    return in_maps


def kernel(dec_out, enc_outs, W_s, W_t, b_t, v_a):
    from concourse.bass_utils import run_bass_kernel_spmd

    nc = _get_nc()
    in_maps = _prep_in_maps(
        {
            "dec_out": dec_out,
            "enc_outs": enc_outs,
            "W_s": W_s,
            "W_t": W_t,
            "b_t": b_t,
            "v_a": v_a,
        }
    )
    res = run_bass_kernel_spmd(nc, in_maps, list(range(N_CORES)))
    return np.concatenate([r["scores"] for r in res.results], axis=1)


# revision 6
# speedup vs baseline: 1.2316x; 1.0995x over previous
"""Bahdanau scorer via separable rank decomposition (Trainium2, Bass/Tile).

scores[t,b,s] = sum_a v_a * tanh(E[a,s] + D[a,t]),  E = W_s enc^T, D = W_t dec^T + b_t

Key trick: the ACT engine's Sin table is only valid for |arg| <~ 3.8, so
tanh(x+y) is expanded as sum_k C_k sin(w_k(x+y)) = sum_k C_k [sin(w_k x)
cos(w_k y) + cos(w_k x) sin(w_k y)] -- a rank-2R separable form -- with
range reduction done in fast f32 DVE ops via the 1.5*2^23 magic-number
round: m = u - round(u) in [-0.5, 0.5], sin(2 pi u) = sin(2 pi m).

scores[t,s] = sum_{k,a} [C_k v_a cos_k(D)[a,t]] * [sin_k(E)[a,s]] + (swap)
is then a PE matmul with contraction 2R*128, fp16 operands, fp32 PSUM
accumulation, landing directly in [t, s] orientation (no output transpose).
Weights are host-pretransposed to fp16; enc/dec transposed by PE+identity.
"""

import numpy as np

SRC, TRG, BATCH, HID, ATT = 256, 256, 32, 512, 128
N_CORES = 8
BC = BATCH // N_CORES

import os as _os

# Fourier fit of tanh(x) on [-9.3, 9.3]: tanh(x) ~= sum_k C[k] sin(FREQ[k] x),
# max abs fit error 7.2e-3 (10 terms, least-squares with minimax reweighting)
FREQ = np.array([0.28049934407051724, 0.5609986881410345, 0.8414980322115516,
                 1.121997376282069, 1.402496720352586, 1.6829960644231032,
                 1.9634954084936205, 2.243994752564138, 2.5244940966346547,
                 2.804993440705172])
C = np.array([1.23413645589884, -0.001074326085440308, 0.3212195767510474,
              0.004232634204585427, 0.11741316647862238, 0.011299951659352471,
              0.03825121652322847, 0.01347067188190923, 0.00909833660163761,
              0.013468026846218053])

_NC_CACHE = {}


def build_nc():
    import concourse.tile as tile
    from concourse import bacc, mybir

    f32 = mybir.dt.float32
    f16 = mybir.dt.float16
    i32 = mybir.dt.int32
    SIN = mybir.ActivationFunctionType.Sin
    TWO_PI = 6.283185307179586
    A = mybir.AluOpType
    TANH = mybir.ActivationFunctionType.Tanh
    SQUARE = mybir.ActivationFunctionType.Square
    EXP = mybir.ActivationFunctionType.Exp
    R = len(C)
    NHB = HID // 128  # 4
    NTB = TRG // 128  # 2

    nc = bacc.Bacc("TRN2", target_bir_lowering=False, debug=False, num_devices=N_CORES)
    dec_in = nc.dram_tensor("dec_out", [TRG, BC, HID], f32, kind="ExternalInput")
    enc_in = nc.dram_tensor("enc_outs", [SRC, BC, HID], f32, kind="ExternalInput")
    id_in = nc.dram_tensor("ident128", [128, 128], f32, kind="ExternalInput")
    wsT_in = nc.dram_tensor("wsT16", [HID, ATT], f16, kind="ExternalInput")
    wtT_in = nc.dram_tensor("wtT16", [HID, ATT], f16, kind="ExternalInput")
    bt_in = nc.dram_tensor("b_t", [ATT, 1], f32, kind="ExternalInput")
    vs_in = nc.dram_tensor("vs", [ATT, R], f32, kind="ExternalInput")
    bE_in = nc.dram_tensor("betasE", [128, R], f32, kind="ExternalInput")
    bD_in = nc.dram_tensor("betasD", [128, R], f32, kind="ExternalInput")
    out = nc.dram_tensor("scores", [TRG, BC, SRC], f32, kind="ExternalOutput")


    with tile.TileContext(nc) as tc:
        with (
            tc.tile_pool(name="consts", bufs=1) as consts,
            tc.tile_pool(name="raw", bufs=4) as rawp,
            tc.tile_pool(name="xt", bufs=3) as xtp,
            tc.tile_pool(name="ed", bufs=1) as ed,
            tc.tile_pool(name="fE", bufs=5) as fEp,
            tc.tile_pool(name="fD", bufs=5) as fDp,
            tc.tile_pool(name="mid", bufs=5) as midp,
            tc.tile_pool(name="oden", bufs=3) as odenp,
            tc.tile_pool(name="ps", bufs=8, space="PSUM") as ps_pool,
        ):
            halfpi = consts.tile([128, 1], f32)
            nc.vector.memset(halfpi[:], 1.5707963267948966)
            ident = consts.tile([128, 128], f32)
            nc.sync.dma_start(out=ident[:], in_=id_in[:])
            # consts + warm the ACT table set off the critical path
            warm = consts.tile([1, 2], f32)
            nc.vector.memset(warm[:], 0.0)
            nc.scalar.activation(warm[:], warm[:], SIN)
            vs_sb = consts.tile([128, R], f32)
            nc.sync.dma_start(out=vs_sb[:], in_=vs_in[:])
            bE_sb = consts.tile([128, R], f32)
            nc.sync.dma_start(out=bE_sb[:], in_=bE_in[:])
            bD_sb = consts.tile([128, R], f32)
            nc.sync.dma_start(out=bD_sb[:], in_=bD_in[:])
            bt_sb = consts.tile([128, 1], f32)
            nc.sync.dma_start(out=bt_sb[:], in_=bt_in[:])
            wT = {}
            for name, w_in in (("s", wsT_in), ("t", wtT_in)):
                wT[name] = consts.tile([128, NHB, 128], f16, name=f"w{name}T")
                for hb in range(NHB):
                    nc.sync.dma_start(
                        out=wT[name][:, hb, :], in_=w_in[hb * 128 : (hb + 1) * 128, :]
                    )

            # --- staging: ED_sb [128a, (E: BC*256 | D: BC*256)] fp32 ---
            ED_sb = ed.tile([128, 2, BC, SRC], f32, name="ED")
            for b in range(BC):
                for name, src_dram, half in (("enc", enc_in, 0), ("dec", dec_in, 1)):
                    xT = xtp.tile([128, NHB, 256], f16, tag="xT", name=f"xT_{b}{name}")
                    for fb in range(2):
                        x_raw = rawp.tile(
                            [128, HID], f32, tag="raw", name=f"raw_{b}{name}{fb}"
                        )
                        nc.sync.dma_start(
                            out=x_raw[:], in_=src_dram[fb * 128 : (fb + 1) * 128, b, :]
                        )
                        for hb in range(NHB):
                            tp = ps_pool.tile(
                                [128, 512], f32, tag="ps", name=f"tp_{b}{name}{fb}{hb}"
                            )
                            nc.tensor.transpose(
                                tp[:, 0:128], x_raw[:, hb * 128 : (hb + 1) * 128], ident[:]
                            )
                            nc.vector.tensor_copy(
                                xT[:, hb, fb * 128 : (fb + 1) * 128], tp[:, 0:128]
                            )
                    w_key = "s" if name == "enc" else "t"
                    pps = ps_pool.tile([128, 512], f32, tag="ps", name=f"pj_{b}{name}")
                    for hb in range(NHB):
                        nc.tensor.matmul(
                            pps[:, 0:256],
                            wT[w_key][:, hb, :],
                            xT[:, hb, :],
                            start=(hb == 0),
                            stop=(hb == NHB - 1),
                        )
                    if name == "enc":
                        nc.scalar.copy(ED_sb[:, half, b, :], pps[:, 0:256])
                    else:
                        nc.scalar.activation(
                            ED_sb[:, half, b, :], pps[:, 0:256],
                            mybir.ActivationFunctionType.Identity,
                            bias=bt_sb[:, 0:1],
                        )

            E_view = ED_sb[:, 0]  # [128, BC, 256]
            D_view = ED_sb[:, 1]

            # --- main loop over components ---
            # one full PSUM bank per (b, tb) region: start=True clears
            # has_written bank-wide, so accumulating regions must not share
            sc_psum = [
                [
                    ps_pool.tile([128, 512], f32, tag="ps", name=f"sc_{b}_{tb}")
                    for tb in range(NTB)
                ]
                for b in range(BC)
            ]
            # --- fourier main loop: R freqs -> 2R components ---
            EDW = 2 * BC * SRC
            for k in range(R):
                ck = float(FREQ[k] / TWO_PI)
                MAGIC = 12582912.0  # 1.5*2^23: fp32 add/sub rounds to nearest int
                # u = c*ED; r1 = round(u); m1 = u - r1  (all fast f32 DVE ops)
                u1 = midp.tile([128, EDW], f32, tag="mf", name=f"u1_{k}")
                nc.vector.tensor_scalar(u1[:], ED_sb[:], ck, None, A.mult)
                r1 = midp.tile([128, EDW], f32, tag="mf", name=f"r1_{k}")
                nc.vector.tensor_scalar(r1[:], u1[:], MAGIC, MAGIC, A.add, A.subtract)
                m1 = midp.tile([128, EDW], f32, tag="mf", name=f"m1_{k}")
                nc.vector.tensor_tensor(m1[:], u1[:], r1[:], A.subtract)
                sinf = fEp.tile([128, EDW], f16, tag="fE", name=f"sin{k}")
                nc.scalar.activation(sinf[:], m1[:], SIN, scale=TWO_PI)
                # cos(2pi*u) = cos(2pi*|m1|) = sin(pi/2 - 2pi*|m1|): args in [-pi/2, pi/2]
                am = midp.tile([128, EDW], f32, tag="mf", name=f"am_{k}")
                nc.scalar.activation(am[:], m1[:], mybir.ActivationFunctionType.Abs)
                cosf = fEp.tile([128, EDW], f16, tag="fE", name=f"cos{k}")
                nc.scalar.activation(cosf[:], am[:], SIN, bias=halfpi[:, 0:1], scale=-TWO_PI)
                # D-side lhsT tiles scaled by a_k*v_a
                fDA = fDp.tile([128, BC * TRG], f16, tag="fD", name=f"fDA{k}")
                nc.vector.tensor_scalar_mul(fDA[:], cosf[:, BC * SRC :], vs_sb[:, k : k + 1])
                fDB = fDp.tile([128, BC * TRG], f16, tag="fD", name=f"fDB{k}")
                nc.vector.tensor_scalar_mul(fDB[:], sinf[:, BC * SRC :], vs_sb[:, k : k + 1])
                for b in range(BC):
                    for tb in range(NTB):
                        nc.tensor.matmul(
                            sc_psum[b][tb][:, 0:256],
                            fDA[:, b * TRG + tb * 128 : b * TRG + (tb + 1) * 128],
                            sinf[:, b * SRC : (b + 1) * SRC],
                            start=(k == 0),
                            stop=False,
                        )
                        nc.tensor.matmul(
                            sc_psum[b][tb][:, 0:256],
                            fDB[:, b * TRG + tb * 128 : b * TRG + (tb + 1) * 128],
                            cosf[:, b * SRC : (b + 1) * SRC],
                            start=False,
                            stop=(k == R - 1),
                        )

            # --- drain ---
            for b in range(BC):
                for tb in range(NTB):
                    ot = odenp.tile([128, SRC], f32, tag="ot", name=f"ot_{b}_{tb}")
                    nc.vector.tensor_copy(ot[:], sc_psum[b][tb][:, 0:256])
                    nc.sync.dma_start(out=out[tb * 128 : (tb + 1) * 128, b, :], in_=ot[:])
    nc.compile()
    return nc


def _get_nc():
    if "nc" not in _NC_CACHE:
        _NC_CACHE["nc"] = build_nc()
    return _NC_CACHE["nc"]


def _prep_in_maps(inputs):
    dec_out = np.ascontiguousarray(np.asarray(inputs["dec_out"], np.float32))
    enc_outs = np.ascontiguousarray(np.asarray(inputs["enc_outs"], np.float32))
    wsT16 = np.ascontiguousarray(np.asarray(inputs["W_s"], np.float32).T.astype(np.float16))
    wtT16 = np.ascontiguousarray(np.asarray(inputs["W_t"], np.float32).T.astype(np.float16))
    b_t = np.asarray(inputs["b_t"], np.float32).reshape(ATT, 1)
    v_a = np.asarray(inputs["v_a"], np.float32).reshape(ATT, 1)

    R = len(C)
    vs = (v_a[:, 0:1] * np.asarray(C, np.float32)[None, :]).astype(np.float32)
    betasE = np.zeros((128, R), np.float32)
    betasD = np.zeros((128, R), np.float32)

    in_maps = []
    for c in range(N_CORES):
        bsl = slice(c * BC, (c + 1) * BC)
        in_maps.append(
            {
                "dec_out": np.ascontiguousarray(dec_out[:, bsl, :]),
                "enc_outs": np.ascontiguousarray(enc_outs[:, bsl, :]),
                "ident128": np.eye(128, dtype=np.float32),
                "wsT16": wsT16,
                "wtT16": wtT16,
                "b_t": b_t,
                "vs": vs,
                "betasE": betasE,
                "betasD": betasD,
            }
        )
    return in_maps


def kernel(dec_out, enc_outs, W_s, W_t, b_t, v_a):
    from concourse.bass_utils import run_bass_kernel_spmd

    nc = _get_nc()
    in_maps = _prep_in_maps(
        {
            "dec_out": dec_out,
            "enc_outs": enc_outs,
            "W_s": W_s,
            "W_t": W_t,
            "b_t": b_t,
            "v_a": v_a,
        }
    )
    res = run_bass_kernel_spmd(nc, in_maps, list(range(N_CORES)))
    return np.concatenate([r["scores"] for r in res.results], axis=1)
